# revision 1
# baseline (speedup 1.0000x reference)
"""BiDirectionalTriangleAttention on 8 TRN2 NeuronCores (Bass/Tile SPMD).

Sharding: I (row) axis of x1/x_pair/mask split across 8 cores (128 rows each).
Per core:
  - triangle bias tri[h, i_loc, j] = einsum(x_pair, wb) computed from a
    host-pre-transposed x_pair shard ([i, c, j] layout, bf16) so the C
    contraction lands on SBUF partitions with zero on-chip transposes of the
    512MB tensor.  Bounced through DRAM to re-layout as [i_part, h, j].
  - mha_1 fully local (queries = local rows, keys = full x2n).
  - mha_2 computed flash-style as a *partial* softmax over the local key rows
    (keys/values = locally updated x1u shard), emitting per-head unnormalized
    o2 partials + exp-sums (ones-augmented V).  Host merges the 8 partials and
    applies the (tiny) gating + output projection + residual for x2u.
"""

import numpy as np
import ml_dtypes

import concourse.bass as bass
import concourse.bacc as bacc
import concourse.mybir as mybir
import concourse.tile as tile
from concourse.bass_utils import run_bass_kernel_spmd

F32 = mybir.dt.float32
BF16 = mybir.dt.bfloat16
BF = ml_dtypes.bfloat16
AX = mybir.AxisListType
ALU = mybir.AluOpType
ACTF = mybir.ActivationFunctionType

B, I, J, C, H, D = 1, 1024, 1024, 128, 8, 32
HD = H * D          # 256
NCORES = 8
IS = I // NCORES    # 128 rows per core
INF = 1e9
EPS = 1e-5
ISCALE = float(1.0 / np.sqrt(np.float32(D)))

IB = 4              # x_pair rows per DMA
SG = 8              # tri rows staged per scratch DMA


def _ln_tile(nc, pool, x, out_dtype, lnw_b, lnb_b, tag):
    """LayerNorm over the free (C) dim of x [P, C] -> new tile [P, C]."""
    P = x.shape[0]
    nsum = pool.tile([P, 1], F32, name=f"nsum_{tag}", tag=f"nsum_{tag}")
    nc.vector.tensor_reduce(nsum, x, axis=AX.X, op=ALU.add, negate=True)
    nc.vector.tensor_scalar_mul(nsum, nsum, 1.0 / C)          # -mu
    xc = pool.tile([P, C], F32, name=f"xc_{tag}", tag=f"xc_{tag}")
    nc.scalar.activation(xc, x, ACTF.Identity, bias=nsum, scale=1.0)  # x - mu
    sq = pool.tile([P, C], F32, name=f"sq_{tag}", tag=f"sq_{tag}")
    vs = pool.tile([P, 1], F32, name=f"vs_{tag}", tag=f"vs_{tag}")
    nc.scalar.activation(sq, xc, ACTF.Square, accum_out=vs)   # sum (x-mu)^2
    sd = pool.tile([P, 1], F32, name=f"sd_{tag}", tag=f"sd_{tag}")
    nc.scalar.activation(sd, vs, ACTF.Sqrt, bias=EPS, scale=1.0 / C)
    rstd = pool.tile([P, 1], F32, name=f"rstd_{tag}", tag=f"rstd_{tag}")
    nc.vector.reciprocal(rstd, sd)
    xn = pool.tile([P, C], F32, name=f"xn_{tag}", tag=f"xn_{tag}")
    nc.scalar.activation(xn, xc, ACTF.Copy, scale=rstd)
    nc.vector.tensor_mul(xn, xn, lnw_b)
    out = pool.tile([P, C], out_dtype, name=f"lnout_{tag}", tag=f"lnout_{tag}")
    nc.vector.tensor_add(out, xn, lnb_b)
    return out


def build_program():
    nc = bacc.Bacc("TRN2", target_bir_lowering=False, debug=False,
                   num_devices=NCORES)

    def din(name, shape, dt=F32):
        return nc.dram_tensor(name, shape, dt, kind="ExternalInput").ap()

    def dout(name, shape, dt=F32):
        return nc.dram_tensor(name, shape, dt, kind="ExternalOutput").ap()

    xpt = din("xpt", [IS, C, J], BF16)     # x_pair shard, [i, c, j] (host-transposed)
    x1s = din("x1s", [IS, C])
    x2d = din("x2d", [J, C])
    msk = din("msk", [IS, J])
    lnw = din("lnw", [128, C])             # row-tiled ln weight
    lnb = din("lnb", [128, C])
    wq1t = din("wq1t", [C, HD])
    wk1t = din("wk1t", [C, HD], BF16)
    wv1t = din("wv1t", [C, HD], BF16)
    wg1t = din("wg1t", [C, HD])
    wo1t = din("wo1t", [HD, C])
    bg1b = din("bg1b", [128, HD])
    bo1c = din("bo1c", [C, 1])
    wq2t = din("wq2t", [C, HD], BF16)
    wk2t = din("wk2t", [C, HD])
    wv2t = din("wv2t", [C, HD])
    wbt = din("wbt", [C, 4 * H], BF16)   # wb.T replicated 4x (col-pack)
    id32 = din("id32", [128, 128])
    idbf = din("idbf", [128, 128], BF16)

    x1u_o = dout("x1u_o", [IS, C])
    o2p_o = dout("o2p_o", [H, D + 1, J])

    with tile.TileContext(nc) as tc:
        cst = tc.alloc_tile_pool(name="cst", bufs=1)
        sb = tc.alloc_tile_pool(name="sb", bufs=1)
        wk = tc.alloc_tile_pool(name="wk", bufs=3)
        xpp = tc.alloc_tile_pool(name="xpp", bufs=4)
        stp = tc.alloc_tile_pool(name="stp", bufs=2)
        drp = tc.alloc_tile_pool(name="drp", bufs=1, space="DRAM")
        ptri = tc.alloc_tile_pool(name="ptri", bufs=2, space="PSUM")
        ptp = tc.alloc_tile_pool(name="ptp", bufs=2, space="PSUM")
        pmm = tc.alloc_tile_pool(name="pmm", bufs=2, space="PSUM")
        pacc = tc.alloc_tile_pool(name="pacc", bufs=2, space="PSUM")

        def load(pool, ap, name, dt=None, bufs=None):
            t = pool.tile(list(ap.shape), dt or ap.dtype, name=name, tag=name,
                          bufs=bufs)
            nc.sync.dma_start(t, ap)
            return t

        # const APs for float biases used by scalar.activation
        for cval in (0.0, EPS):
            cap = cst.tile([128, 1], F32, name=f"constap_{cval}",
                           tag=f"constap_{cval}")
            nc.vector.memset(cap, cval)
            nc.const_aps.aps[(F32, cval)] = cap

        # ---- constants / weights ----
        c_id32 = load(cst, id32, "c_id32")
        c_idbf = load(cst, idbf, "c_idbf")
        c_lnw = load(cst, lnw, "c_lnw")
        c_lnb = load(cst, lnb, "c_lnb")
        c_wq1t = load(cst, wq1t, "c_wq1t")
        c_wk1t = load(cst, wk1t, "c_wk1t")
        c_wv1t = load(cst, wv1t, "c_wv1t")
        c_wg1t = load(cst, wg1t, "c_wg1t")
        c_wo1t = cst.tile([128, 2, C], F32, name="c_wo1t", tag="c_wo1t")
        nc.sync.dma_start(c_wo1t, wo1t.rearrange("(t p) c -> p t c", p=128))
        c_bg1b = load(cst, bg1b, "c_bg1b")
        c_bo1c = load(cst, bo1c, "c_bo1c")
        c_wq2t = load(cst, wq2t, "c_wq2t")
        c_wk2t = load(cst, wk2t, "c_wk2t")
        c_wv2t = load(cst, wv2t, "c_wv2t")
        c_wbt = load(cst, wbt, "c_wbt")

        # ---- small inputs + LN ----
        t_x1 = load(sb, x1s, "t_x1")
        t_msk = load(sb, msk, "t_msk")

        # ---- triangle bias ----
        # 4 rows (i) per matmul group via tile_position col-packing: row i0+k's
        # [8, 512] output lands at psum partition base 32k, so the PSUM->SBUF
        # copy runs full-lane [128, 512].  Staged to DRAM scratch [g, p, j]
        # (rows 8-31 of each 32-group are garbage, skipped on reload).
        NG = IS // IB                       # 32 groups of 4 rows
        GS = 4                              # groups staged per scratch DMA
        tri_scr = drp.tile([NG, 128, J], BF16, name="tri_scr", tag="tri_scr")
        for g in range(NG):
            i0 = g * IB
            xt = xpp.tile([C, IB, J], BF16, name="xt", tag="xt")
            nc.sync.dma_start(xt, xpt[i0:i0 + IB].rearrange("i c j -> c i j"))
            if g % GS == 0:
                stg = stp.tile([128, GS, J], BF16, name="stg", tag="stg")
            for blk in range(2):
                ps = ptri.tile([128, 512], F32, name="ps_tri", tag="tri")
                for k in range(IB):
                    nc.tensor.matmul(ps[32 * k:32 * (k + 1), :], c_wbt,
                                     xt[:, k, blk * 512:(blk + 1) * 512],
                                     start=True, stop=True,
                                     tile_position=(0, 32 * k))
                dst = stg[:, g % GS, blk * 512:(blk + 1) * 512]
                if (g + blk) % 2 == 0:
                    nc.vector.tensor_copy(dst, ps)
                else:
                    nc.scalar.copy(dst, ps)
            if g % GS == GS - 1:
                nc.sync.dma_start(
                    tri_scr[g - GS + 1:g + 1].rearrange("g p j -> p g j"), stg)

        # ---- LN + projections (fill stream-tail gap) ----

        x1n = _ln_tile(nc, sb, t_x1, F32, c_lnw, c_lnb, "x1")
        tp = ptp.tile([128, 128], F32, name="tp_x1n", tag="tp")
        nc.tensor.transpose(tp, x1n, c_id32)
        x1nT = sb.tile([128, IS], F32, name="x1nT", tag="x1nT")
        nc.vector.tensor_copy(x1nT, tp)

        x2nT = sb.tile([128, J], BF16, name="x2nT", tag="x2nT")
        for jt in range(8):
            x2t = load(wk, x2d[jt * 128:(jt + 1) * 128, :], "x2t")
            x2n_jt = _ln_tile(nc, wk, x2t, BF16, c_lnw, c_lnb, "x2")
            tpb = ptp.tile([128, 128], BF16, name="tp_x2n", tag="tp")
            nc.tensor.transpose(tpb, x2n_jt, c_idbf)
            nc.vector.tensor_copy(x2nT[:, jt * 128:(jt + 1) * 128], tpb)

        # mask bias  mb = INF * (mask - 1)
        mb = sb.tile([IS, J], F32, name="mb", tag="mb")
        nc.scalar.activation(mb, t_msk, ACTF.Copy, bias=-INF, scale=INF)

        # ---- projections ----
        # q1T/k1T per head at partition base 0 (lhsT = per-head weight slice)
        q1T = sb.tile([D, H, IS], BF16, name="q1T", tag="q1T")
        k1T = sb.tile([D, H, J], BF16, name="k1T", tag="k1T")
        for h in range(H):
            hs = slice(h * D, (h + 1) * D)
            qp = pmm.tile([D, IS], F32, name="qp1", tag="mm")
            nc.tensor.matmul(qp, c_wq1t[:, hs], x1nT, start=True, stop=True)
            nc.scalar.activation(q1T[:, h, :], qp, ACTF.Copy, scale=ISCALE)
            for blk in range(2):
                kp = pmm.tile([D, 512], F32, name="kp1", tag="mm")
                nc.tensor.matmul(kp, c_wk1t[:, hs],
                                 x2nT[:, blk * 512:(blk + 1) * 512],
                                 start=True, stop=True)
                if h % 2 == 0:
                    nc.scalar.copy(k1T[:, h, blk * 512:(blk + 1) * 512], kp)
                else:
                    nc.vector.tensor_copy(k1T[:, h, blk * 512:(blk + 1) * 512], kp)

        # v1 [j, hd] (bf16) per j-tile
        v1 = sb.tile([128, 8, HD], BF16, name="v1", tag="v1")
        for jt in range(8):
            vp = pmm.tile([128, HD], F32, name="vp1", tag="mm")
            nc.tensor.matmul(vp, x2nT[:, jt * 128:(jt + 1) * 128], c_wv1t,
                             start=True, stop=True)
            nc.vector.tensor_copy(v1[:, jt, :], vp)

        # gating g1 = sigmoid(x1n @ wg1.T + bg1)   [i, hd]
        gp = pmm.tile([IS, HD], F32, name="gp1", tag="mm")
        nc.tensor.matmul(gp, x1nT, c_wg1t, start=True, stop=True)
        g1 = sb.tile([IS, HD], F32, name="g1", tag="g1")
        nc.vector.tensor_add(g1, gp, c_bg1b)
        nc.scalar.activation(g1, g1, ACTF.Sigmoid)

        q2T = sb.tile([D, H, J], BF16, name="q2T", tag="q2T")
        for h in range(H):
            hs = slice(h * D, (h + 1) * D)
            for blk in range(2):
                qp2 = pmm.tile([D, 512], F32, name="qp2", tag="mm")
                nc.tensor.matmul(qp2, c_wq2t[:, hs],
                                 x2nT[:, blk * 512:(blk + 1) * 512],
                                 start=True, stop=True)
                if h % 2 == 0:
                    nc.scalar.activation(q2T[:, h, blk * 512:(blk + 1) * 512],
                                         qp2, ACTF.Copy, scale=ISCALE)
                else:
                    nc.vector.tensor_scalar_mul(
                        q2T[:, h, blk * 512:(blk + 1) * 512], qp2, ISCALE)

        # reload per head as [i_part, j] (row 32k+h of group g -> i=4g+k)
        # and add mask bias -> combined bias per head
        _scr_r = tri_scr.rearrange("g (k r) j -> (g k) r j", k=IB)
        tribs = []
        for h in range(H):
            th = sb.tile([IS, J], BF16, name=f"trib{h}", tag=f"trib{h}")
            nc.sync.dma_start(th, _scr_r[:, h, :])
            nc.vector.tensor_add(th, th, mb)
            tribs.append(th)

        # ---- mha_1 ----
        l1 = sb.tile([IS, H], F32, name="l1", tag="l1")
        r1 = sb.tile([IS, H], F32, name="r1", tag="r1")
        o1n = sb.tile([IS, HD], F32, name="o1n", tag="o1n")
        for h in range(H):
            p1 = wk.tile([IS, J], BF16, name="p1", tag="p1")
            l1p = wk.tile([IS, 2], F32, name="l1p", tag="l1p")
            for blk in range(2):
                bs = slice(blk * 512, (blk + 1) * 512)
                sp = ptri.tile([IS, 512], F32, name="sp1", tag="tri")
                nc.tensor.matmul(sp, c_idbf, tribs[h][:, bs],
                                 start=True, stop=False)
                nc.tensor.matmul(sp, q1T[:, h, :], k1T[:, h, bs],
                                 start=False, stop=True)
                nc.scalar.activation(p1[:, bs], sp, ACTF.Exp,
                                     accum_out=l1p[:, blk:blk + 1])
            nc.vector.tensor_reduce(l1[:, h:h + 1], l1p, axis=AX.X, op=ALU.add)
            nc.vector.reciprocal(r1[:, h:h + 1], l1[:, h:h + 1])
            p1T = wk.tile([128, 8, IS], BF16, name="p1T", tag="p1T")
            for jt in range(8):
                tpb = ptp.tile([128, 128], BF16, name="tp_p1", tag="tp")
                nc.tensor.transpose(tpb, p1[:, jt * 128:(jt + 1) * 128], c_idbf)
                if jt % 2 == 0:
                    nc.vector.tensor_copy(p1T[:, jt, :], tpb)
                else:
                    nc.scalar.copy(p1T[:, jt, :], tpb)
            op = pacc.tile([IS, D], F32, name="op1", tag="acc")
            for jt in range(8):
                nc.tensor.matmul(op, p1T[:, jt, :], v1[:, jt, h * D:(h + 1) * D],
                                 start=(jt == 0), stop=(jt == 7))
            nc.scalar.activation(o1n[:, h * D:(h + 1) * D], op, ACTF.Copy,
                                 scale=r1[:, h:h + 1])

        og = sb.tile([IS, HD], F32, name="og", tag="og")
        nc.vector.tensor_mul(og, o1n, g1)
        ogT = sb.tile([128, 2, IS], F32, name="ogT", tag="ogT")
        for t in range(2):
            tp2 = ptp.tile([128, 128], F32, name="tp_og", tag="tp")
            nc.tensor.transpose(tp2, og[:, t * 128:(t + 1) * 128], c_id32)
            nc.vector.tensor_copy(ogT[:, t, :], tp2)

        xop = pacc.tile([C, IS], F32, name="xop", tag="acc")
        for t in range(2):
            nc.tensor.matmul(xop, c_wo1t[:, t, :], ogT[:, t, :],
                             start=(t == 0), stop=(t == 1))
        x1uT = sb.tile([C, IS], F32, name="x1uT", tag="x1uT")
        nc.scalar.activation(x1uT, xop, ACTF.Identity, bias=c_bo1c)
        nc.vector.tensor_add(x1uT, x1uT, x1nT)

        # x1u shard out (untransposed)
        tpo = ptp.tile([128, 128], F32, name="tp_x1u", tag="tp")
        nc.tensor.transpose(tpo, x1uT, c_id32)
        x1u_sb = sb.tile([IS, C], F32, name="x1u_sb", tag="x1u_sb")
        nc.vector.tensor_copy(x1u_sb, tpo)
        nc.sync.dma_start(x1u_o, x1u_sb)

        # ---- mha_2 partials over local keys ----
        k2T = sb.tile([D, H, IS], BF16, name="k2T", tag="k2T")
        for h in range(H):
            hs = slice(h * D, (h + 1) * D)
            kp2 = pmm.tile([D, IS], F32, name="kp2", tag="mm")
            nc.tensor.matmul(kp2, c_wk2t[:, hs], x1uT, start=True, stop=True)
            nc.scalar.copy(k2T[:, h, :], kp2)

        v2p = pmm.tile([IS, HD], F32, name="v2p", tag="mm")
        nc.tensor.matmul(v2p, x1uT, c_wv2t, start=True, stop=True)
        v2a = sb.tile([IS, H, D + 1], BF16, name="v2a", tag="v2a")
        nc.vector.memset(v2a, 1.0)
        for h in range(H):
            nc.vector.tensor_copy(v2a[:, h, :D], v2p[:, h * D:(h + 1) * D])

        for h in range(H):
            p2 = wk.tile([IS, J], BF16, name="p2", tag="p1")
            for blk in range(2):
                bs = slice(blk * 512, (blk + 1) * 512)
                sp2 = ptri.tile([IS, 512], F32, name="sp2", tag="tri")
                nc.tensor.matmul(sp2, c_idbf, tribs[h][:, bs],
                                 start=True, stop=False)
                nc.tensor.matmul(sp2, k2T[:, h, :], q2T[:, h, bs],
                                 start=False, stop=True)
                nc.scalar.activation(p2[:, bs], sp2, ACTF.Exp)
            o2h = wk.tile([D + 1, J], F32, name="o2h", tag="o2h")
            for blk in range(2):
                o2ps = pmm.tile([D + 1, 512], F32, name="o2ps", tag="mm")
                nc.tensor.matmul(o2ps, v2a[:, h, :],
                                 p2[:, blk * 512:(blk + 1) * 512],
                                 start=True, stop=True)
                if blk == 0:
                    nc.vector.tensor_copy(o2h[:, :512], o2ps)
                else:
                    nc.scalar.copy(o2h[:, 512:], o2ps)
            nc.sync.dma_start(o2p_o[h], o2h)

        for p in reversed((cst, sb, wk, xpp, stp, drp, ptri, ptp, pmm, pacc)):
            p.release()

    nc.compile()
    return nc


_CACHE = {}


def _get_program():
    if "nc" not in _CACHE:
        _CACHE["nc"] = build_program()
    return _CACHE["nc"]


def _np_ln(x):
    mu = x.mean(-1, keepdims=True)
    var = np.square(x - mu).mean(-1, keepdims=True)
    return (x - mu) / np.sqrt(var + EPS)


def make_in_maps(x1, x2, x_pair, mask, ln_w, ln_b, wb,
                 wq1, wk1, wv1, wg1, bg1, wo1, bo1,
                 wq2, wk2, wv2, wg2, bg2, wo2, bo2):
    f = np.float32
    shared = {
        "x2d": np.ascontiguousarray(x2[0], dtype=f),
        "lnw": np.tile(np.asarray(ln_w, f), (128, 1)),
        "lnb": np.tile(np.asarray(ln_b, f), (128, 1)),
        "wq1t": np.ascontiguousarray(np.asarray(wq1, f).T),
        "wk1t": np.ascontiguousarray(np.asarray(wk1, f).T).astype(BF),
        "wv1t": np.ascontiguousarray(np.asarray(wv1, f).T).astype(BF),
        "wg1t": np.ascontiguousarray(np.asarray(wg1, f).T),
        "wo1t": np.ascontiguousarray(np.asarray(wo1, f).T),
        "bg1b": np.tile(np.asarray(bg1, f), (128, 1)),
        "bo1c": np.asarray(bo1, f)[:, None].copy(),
        "wq2t": np.ascontiguousarray(np.asarray(wq2, f).T).astype(BF),
        "wk2t": np.ascontiguousarray(np.asarray(wk2, f).T),
        "wv2t": np.ascontiguousarray(np.asarray(wv2, f).T),
        "wbt": np.tile(np.ascontiguousarray(np.asarray(wb, f).T), (1, 4)).astype(BF),
        "id32": np.eye(128, dtype=f),
        "idbf": np.eye(128, dtype=f).astype(BF),
    }
    in_maps = []
    x1np = np.asarray(x1, f)
    xpnp = np.asarray(x_pair, f)
    msknp = np.asarray(mask, f)
    for m in range(NCORES):
        sl = slice(m * IS, (m + 1) * IS)
        im = dict(shared)
        im["x1s"] = np.ascontiguousarray(x1np[0, sl])
        im["msk"] = np.ascontiguousarray(msknp[0, sl])
        im["xpt"] = np.ascontiguousarray(
            xpnp[0, sl].transpose(0, 2, 1)).astype(BF)
        in_maps.append(im)
    return in_maps


def combine(results, x2, wg2, bg2, wo2, bo2):
    f = np.float32
    x1u = np.concatenate([results[m]["x1u_o"] for m in range(NCORES)],
                         axis=0)[None]
    o2p = np.sum([results[m]["o2p_o"].astype(np.float64)
                  for m in range(NCORES)], axis=0)
    o2 = o2p[:, :D, :]                    # [H, D, J]
    l2 = o2p[:, D, :]                     # [H, J]
    on = (o2 / l2[:, None, :]).astype(f)
    o_fl = on.transpose(2, 0, 1).reshape(J, HD)       # [j, hd]
    x2n = _np_ln(np.asarray(x2[0], f))
    g2 = 1.0 / (1.0 + np.exp(-(x2n @ np.asarray(wg2, f).T
                               + np.asarray(bg2, f))))
    x2u = x2n + (o_fl * g2) @ np.asarray(wo2, f).T + np.asarray(bo2, f)
    return x1u.astype(f), x2u[None].astype(f)


def kernel(**inputs):
    nc = _get_program()
    in_maps = make_in_maps(**inputs)
    res = run_bass_kernel_spmd(nc, in_maps, core_ids=list(range(NCORES)))
    return combine(res.results, inputs["x2"], inputs["wg2"], inputs["bg2"],
                   inputs["wo2"], inputs["bo2"])


if __name__ == "__main__":
    import reference
    inputs = {k: np.asarray(v) for k, v in reference.setup_inputs().items()}
    e1, e2 = reference.reference(**inputs)
    a1, a2 = kernel(**inputs)
    for name, e, a in (("x1u", e1, a1), ("x2u", e2, a2)):
        e = np.asarray(e)
        err = np.abs(a - e).max() / (np.abs(e).max() + 1e-12)
        print(f"{name}: rel_err={err:.3e}")

